# revision 4
# baseline (speedup 1.0000x reference)
"""AttnBlock (GroupNorm + 1x1-conv self-attention + residual) on 8 trn2 cores.

Sharding: pure data parallel -- batch b runs on core b (B=8, n_cores=8).
Each core computes its whole batch: GN -> QKV -> attention -> proj -> residual.

Layout choices (per core, C=512, N=H*W=4096):
  h, Q, K stored [C, N] (channels on partitions, 4 tiles of 128)
  V stored [N, C] (positions on partitions, 32 tiles of 128)
  S^T = K^T Q computed [keys m, queries n]; softmax over m = partition dim
  across 32 m-tiles: denominator via DVE accumulation + gpsimd
  partition_all_reduce; division deferred to after the output projection
  (out = x + (wp @ A_unnorm) / D).
  => no transposes anywhere in the attention.

All matmuls take fp16 inputs (1 PE cycle/row vs 4 for fp32) and accumulate in
fp32 PSUM. Host-side numpy emulation: rel_l2 error vs fp32 reference ~ 3.6e-5.
"""

import sys

sys.path.insert(0, "/opt/trn_rl_repo")

import numpy as np

import concourse.bass as bass
from concourse import bacc
import concourse.tile as tile
from concourse import bass_isa, mybir
from concourse.bass import ds
from concourse.bass_utils import run_bass_kernel_spmd

P = 128
C = 512
CT = C // P          # 4 channel tiles
N = 4096
NCH = N // 512       # 8 column chunks of 512
NT = N // P          # 32 position tiles of 128
G = 32               # groups
GS = C // G          # 16 channels per group
EPS = 1e-5
SCALE = float(C) ** -0.5

F16 = mybir.dt.float16
F32 = mybir.dt.float32

TUNE = dict(
    work_bufs=2,
    acc_bufs=4,
    s_bufs=3,
    es_bufs=4,
    xch_bufs=2,
    asb_bufs=5,
    ot_bufs=3,
)


def build_nc(with_cb: bool, tune=None):
    t_ = dict(TUNE)
    if tune:
        t_.update(tune)
    nc = bacc.Bacc(trn_type="TRN2")

    x_d = nc.dram_tensor("x", [C, N], F32, kind="ExternalInput")
    wq_d = nc.dram_tensor("wqT", [C, C], F16, kind="ExternalInput")
    wk_d = nc.dram_tensor("wkT", [C, C], F16, kind="ExternalInput")
    wv_d = nc.dram_tensor("wvT", [C, C], F16, kind="ExternalInput")
    wp_d = nc.dram_tensor("wpT", [C, C], F16, kind="ExternalInput")
    bqk_d = nc.dram_tensor("bqk", [P, 2, CT], F32, kind="ExternalInput")
    gnw_d = nc.dram_tensor("gnw", [P, CT, 2], F32, kind="ExternalInput")
    gstat_d = nc.dram_tensor("gstat", [P, CT, G], F32, kind="ExternalInput")
    gexp_d = nc.dram_tensor("gexp", [P, CT, P], F32, kind="ExternalInput")
    if with_cb:
        cb_d = nc.dram_tensor("cb", [P, CT], F32, kind="ExternalInput")
    out_d = nc.dram_tensor("out", [C, N], F32, kind="ExternalOutput")

    x3 = x_d.rearrange("(t p) n -> p t n", p=P)
    o3 = out_d.rearrange("(t p) n -> p t n", p=P)
    w3 = {
        "q": wq_d.rearrange("(t p) o -> p t o", p=P),
        "k": wk_d.rearrange("(t p) o -> p t o", p=P),
        "v": wv_d.rearrange("(t p) o -> p t o", p=P),
        "p": wp_d.rearrange("(t p) o -> p t o", p=P),
    }

    from contextlib import ExitStack

    with tile.TileContext(nc) as tc, ExitStack() as ctx:
        persist = ctx.enter_context(tc.tile_pool(name="persist", bufs=1))
        work = ctx.enter_context(tc.tile_pool(name="work", bufs=t_["work_bufs"]))
        ps_acc = ctx.enter_context(
            tc.tile_pool(name="ps_acc", bufs=t_["acc_bufs"], space="PSUM")
        )
        ps_s = ctx.enter_context(
            tc.tile_pool(name="ps_s", bufs=t_["s_bufs"], space="PSUM")
        )
        ps_gn = ctx.enter_context(tc.tile_pool(name="ps_gn", bufs=1, space="PSUM"))

        # ---------------- persistent weights / constants ----------------
        wsb = {}
        for kk in ("q", "k", "v", "p"):
            wsb[kk] = persist.tile([P, CT, C], F16, tag=f"w{kk}", name=f"w{kk}_sb")
            nc.sync.dma_start(out=wsb[kk], in_=w3[kk])
        bqk_sb = persist.tile([P, 2, CT], F32, tag="bqk")
        nc.sync.dma_start(out=bqk_sb, in_=bqk_d.ap())
        gnw_sb = persist.tile([P, CT, 2], F32, tag="gnw")
        nc.sync.dma_start(out=gnw_sb, in_=gnw_d.ap())
        gstat_sb = persist.tile([P, CT, G], F32, tag="gstat")
        nc.sync.dma_start(out=gstat_sb, in_=gstat_d.ap())
        gexp_sb = persist.tile([P, CT, P], F32, tag="gexp")
        nc.sync.dma_start(out=gexp_sb, in_=gexp_d.ap())
        if with_cb:
            cb_sb = persist.tile([P, CT], F32, tag="cb")
            nc.sync.dma_start(out=cb_sb, in_=cb_d.ap())
        eps_sb = persist.tile([G, 1], F32, tag="eps")
        nc.vector.memset(eps_sb, EPS)

        Q_sb = persist.tile([P, CT, N], F16, tag="Q")
        K_sb = persist.tile([P, CT, N], F16, tag="K")
        V_sb = persist.tile([P, NT, C], F16, tag="V")

        SD = nc.vector.BN_STATS_DIM
        AD = nc.vector.BN_AGGR_DIM

        # ---------------- phase 1: GN statistics (stream x) ----------------
        stats = persist.tile([P, CT, NCH, SD], F32, tag="stats")
        for nch in range(NCH):
            xch = work.tile(
                [P, CT, 512], F32, tag="xch", bufs=t_["xch_bufs"], name="xch"
            )
            nc.sync.dma_start(out=xch, in_=x3[:, :, ds(nch * 512, 512)])
            for t in range(CT):
                nc.vector.bn_stats(out=stats[:, t, nch, :], in_=xch[:, t, :])
        mv = persist.tile([P, CT, AD], F32, tag="mv")
        rs = persist.tile([P, CT, 2], F32, tag="rs")
        for t in range(CT):
            nc.vector.bn_aggr(out=mv[:, t, :], in_=stats[:, t, :, :])
            # rs = [mean, E[x^2]] per channel
            nc.vector.tensor_mul(rs[:, t, 1:2], mv[:, t, 0:1], mv[:, t, 0:1])
            nc.vector.tensor_add(rs[:, t, 1:2], rs[:, t, 1:2], mv[:, t, 1:2])
            nc.vector.tensor_copy(rs[:, t, 0:1], mv[:, t, 0:1])

        # group reduce: pstat[g, :] = [mu_g, E2_g]  (1/GS folded into gstat)
        pstat = ps_gn.tile([G, 2], F32, tag="psgn", name="pstat")
        for t in range(CT):
            nc.tensor.matmul(
                pstat,
                lhsT=gstat_sb[:, t, :],
                rhs=rs[:, t, :],
                start=(t == 0),
                stop=(t == CT - 1),
            )
        gmr = persist.tile([P, 2], F32, tag="gmr")  # [mu_g, rstd_g], zero padded
        nc.vector.memset(gmr, 0.0)
        gst = persist.tile([G, 2], F32, tag="gst")  # pstat evacuated to SBUF
        nc.vector.tensor_copy(gst, pstat)
        gvar = persist.tile([G, 1], F32, tag="gvar")
        nc.vector.tensor_copy(gmr[:G, 0:1], gst[:, 0:1])
        nc.vector.tensor_mul(gvar, gst[:, 0:1], gst[:, 0:1])
        nc.vector.tensor_tensor(
            gvar, gst[:, 1:2], gvar, op=mybir.AluOpType.subtract
        )
        nc.scalar.activation(
            gmr[:G, 1:2], gvar, mybir.ActivationFunctionType.Sqrt, bias=eps_sb
        )
        nc.vector.reciprocal(gmr[:G, 1:2], gmr[:G, 1:2])

        # expand to channels, then per-channel affine A,B:
        # h = A*x + B with A = gamma*rstd, B = beta - mu*A
        AB = persist.tile([P, CT, 2], F32, tag="AB")
        for t in range(CT):
            pexp = ps_gn.tile([P, 2], F32, tag="psgn", name="pexp")
            nc.tensor.matmul(
                pexp, lhsT=gexp_sb[:, t, :], rhs=gmr, start=True, stop=True
            )
            nc.vector.tensor_mul(AB[:, t, 0:1], gnw_sb[:, t, 0:1], pexp[:, 1:2])
            nc.vector.tensor_mul(AB[:, t, 1:2], pexp[:, 0:1], AB[:, t, 0:1])
            nc.vector.tensor_tensor(
                AB[:, t, 1:2],
                gnw_sb[:, t, 1:2],
                AB[:, t, 1:2],
                op=mybir.AluOpType.subtract,
            )

        # ------------- phase 2+3: normalize + QKV, per 512-col chunk -------------
        for nch in range(NCH):
            xch = work.tile(
                [P, CT, 512], F32, tag="xch", bufs=t_["xch_bufs"], name="xch"
            )
            nc.sync.dma_start(out=xch, in_=x3[:, :, ds(nch * 512, 512)])
            hch = work.tile([P, CT, 512], F16, tag="hch", name="hch")
            for t in range(CT):
                nc.vector.tensor_scalar(
                    out=hch[:, t, :],
                    in0=xch[:, t, :],
                    scalar1=AB[:, t, 0:1],
                    scalar2=AB[:, t, 1:2],
                    op0=mybir.AluOpType.mult,
                    op1=mybir.AluOpType.add,
                )
            for bcol, (kk, dst) in enumerate((("q", Q_sb), ("k", K_sb))):
                for o in range(CT):
                    pq = ps_acc.tile([P, 512], F32, tag="acc", name="pq")
                    for t in range(CT):
                        nc.tensor.matmul(
                            pq,
                            lhsT=wsb[kk][:, t, ds(o * P, P)],
                            rhs=hch[:, t, :],
                            start=(t == 0),
                            stop=(t == CT - 1),
                        )
                    nc.scalar.activation(
                        dst[:, o, ds(nch * 512, 512)],
                        pq,
                        mybir.ActivationFunctionType.Identity,
                        bias=bqk_sb[:, bcol, o : o + 1],
                    )
            for j in range(4):
                m = nch * 4 + j
                pv = ps_acc.tile([P, 512], F32, tag="acc", name="pv")
                for t in range(CT):
                    nc.tensor.matmul(
                        pv,
                        lhsT=hch[:, t, ds(j * P, P)],
                        rhs=wsb["v"][:, t, :],
                        start=(t == 0),
                        stop=(t == CT - 1),
                    )
                nc.scalar.copy(V_sb[:, m, :], pv)

        # ------------- phase 4: attention + projection, per query block -------------
        for nb in range(NCH):
            dacc = work.tile([P, 512], F32, tag="dacc", name="dacc")
            nc.vector.memset(dacc, 0.0)
            pa = [
                ps_acc.tile([P, 512], F32, tag="acc", name=f"pa{c}")
                for c in range(CT)
            ]
            es_tiles = [None] * NT
            for m in range(NT):
                # scores^T tile: S[m-tile, nb chunk]
                ps = ps_s.tile([P, 512], F32, tag="s", name="ps")
                for t in range(CT):
                    nc.tensor.matmul(
                        ps,
                        lhsT=K_sb[:, t, ds(m * P, P)],
                        rhs=Q_sb[:, t, ds(nb * 512, 512)],
                        start=(t == 0),
                        stop=(t == CT - 1),
                    )
                # software pipeline: PV for m-1 queued before exp(m) so the PE
                # never waits on the ACT engine
                if m > 0:
                    esp = es_tiles[m - 1]
                    for c in range(CT):
                        nc.tensor.matmul(
                            pa[c],
                            lhsT=V_sb[:, m - 1, ds(c * P, P)],
                            rhs=esp,
                            start=(m == 1),
                            stop=False,
                        )
                es = work.tile([P, 512], F16, tag="es", bufs=t_["es_bufs"], name="es")
                nc.scalar.activation(
                    es, ps, mybir.ActivationFunctionType.Exp, scale=SCALE
                )
                nc.vector.tensor_add(dacc, dacc, es)
                es_tiles[m] = es
            esp = es_tiles[NT - 1]
            for c in range(CT):
                nc.tensor.matmul(
                    pa[c],
                    lhsT=V_sb[:, NT - 1, ds(c * P, P)],
                    rhs=esp,
                    start=False,
                    stop=True,
                )

            dbc = work.tile([P, 512], F32, tag="dbc", name="dbc")
            nc.gpsimd.partition_all_reduce(
                dbc, dacc, channels=P, reduce_op=bass_isa.ReduceOp.add
            )
            rd = work.tile([P, 512], F32, tag="rd", name="rd")
            nc.vector.reciprocal(rd, dbc)

            asb = []
            for c in range(CT):
                a = work.tile(
                    [P, 512], F16, tag="asb", bufs=t_["asb_bufs"], name=f"asb{c}"
                )
                nc.scalar.copy(a, pa[c])
                asb.append(a)

            xch = work.tile(
                [P, CT, 512], F32, tag="xch", bufs=t_["xch_bufs"], name="xch"
            )
            nc.sync.dma_start(out=xch, in_=x3[:, :, ds(nb * 512, 512)])
            for o in range(CT):
                po = ps_acc.tile([P, 512], F32, tag="acc", name="po")
                for c in range(CT):
                    nc.tensor.matmul(
                        po,
                        lhsT=wsb["p"][:, c, ds(o * P, P)],
                        rhs=asb[c],
                        start=(c == 0),
                        stop=(c == CT - 1),
                    )
                ot = work.tile([P, 512], F32, tag="ot", bufs=t_["ot_bufs"], name="ot")
                nc.vector.tensor_mul(ot, po, rd)
                if with_cb:
                    nc.vector.tensor_scalar(
                        out=ot,
                        in0=ot,
                        scalar1=cb_sb[:, o : o + 1],
                        op0=mybir.AluOpType.add,
                    )
                nc.vector.tensor_add(ot, ot, xch[:, o, :])
                nc.sync.dma_start(out=o3[:, o, ds(nb * 512, 512)], in_=ot)

    nc.compile()
    return nc


_NC_CACHE = {}


def get_nc(with_cb: bool, tune=None):
    key = (with_cb, tuple(sorted((tune or {}).items())))
    if key not in _NC_CACHE:
        _NC_CACHE[key] = build_nc(with_cb, tune)
    return _NC_CACHE[key]


def make_in_maps(x, gn_gamma, gn_beta, wq, bq, wk, bk, wv, bv, wp, bp):
    """Host-side prep: returns (in_maps list for 8 cores, with_cb flag)."""
    x = np.asarray(x, dtype=np.float32)
    B = x.shape[0]
    assert x.shape == (8, C, 64, 64)

    f32 = np.float32
    wqT = np.ascontiguousarray(np.asarray(wq, f32).T).astype(np.float16)
    wkT = np.ascontiguousarray(np.asarray(wk, f32).T).astype(np.float16)
    wvT = np.ascontiguousarray(np.asarray(wv, f32).T).astype(np.float16)
    wpT = np.ascontiguousarray(np.asarray(wp, f32).T).astype(np.float16)

    bq = np.asarray(bq, f32)
    bk = np.asarray(bk, f32)
    bqk = np.ascontiguousarray(
        np.stack([bq.reshape(CT, P).T, bk.reshape(CT, P).T], axis=1)
    )  # [P, 2, CT]
    gnw = np.ascontiguousarray(
        np.stack(
            [np.asarray(gn_gamma, f32).reshape(CT, P).T,
             np.asarray(gn_beta, f32).reshape(CT, P).T],
            axis=2,
        )
    )  # [P, CT, 2]

    gstat = np.zeros((P, CT, G), f32)
    for t in range(CT):
        for p in range(P):
            gstat[p, t, 8 * t + p // GS] = 1.0 / GS
    gexp = np.zeros((P, CT, P), f32)  # [g(padded to 128), t, c]
    for t in range(CT):
        for c in range(P):
            gexp[8 * t + c // GS, t, c] = 1.0

    cb = (np.asarray(wp, f32) @ np.asarray(bv, f32) + np.asarray(bp, f32)).astype(f32)
    with_cb = bool(np.abs(cb).max() > 0)
    cb4 = np.ascontiguousarray(cb.reshape(CT, P).T)  # [P, CT]

    shared = {
        "wqT": wqT, "wkT": wkT, "wvT": wvT, "wpT": wpT,
        "bqk": bqk, "gnw": gnw, "gstat": gstat, "gexp": gexp,
    }
    if with_cb:
        shared["cb"] = cb4

    in_maps = []
    for b in range(B):
        m = dict(shared)
        m["x"] = np.ascontiguousarray(x[b].reshape(C, N))
        in_maps.append(m)
    return in_maps, with_cb


def kernel(x, gn_gamma, gn_beta, wq, bq, wk, bk, wv, bv, wp, bp):
    in_maps, with_cb = make_in_maps(
        x, gn_gamma, gn_beta, wq, bq, wk, bk, wv, bv, wp, bp
    )
    nc = get_nc(with_cb)
    res = run_bass_kernel_spmd(nc, in_maps, core_ids=list(range(8)))
    outs = [res.results[b]["out"].reshape(C, 64, 64) for b in range(8)]
    return np.stack(outs).astype(np.float32)


# revision 9
# speedup vs baseline: 1.7838x; 1.7838x over previous
"""AttnBlock (GroupNorm + 1x1-conv self-attention + residual) on 8 trn2 cores.

Sharding: pure data parallel -- batch b runs on core b (B=8, n_cores=8).
Each core computes its whole batch: GN -> QKV -> attention -> proj -> residual.

Layout choices (per core, C=512, N=H*W=4096):
  h, Q, K stored [C, N] (channels on partitions, 4 tiles of 128)
  V stored [N, C] (positions on partitions, 32 tiles of 128)
  S^T = K^T Q computed [keys m, queries n]; softmax over m = partition dim
  across 32 m-tiles: denominator via DVE accumulation + gpsimd
  partition_all_reduce; division deferred to after the output projection
  (out = x + (wp @ A_unnorm) / D).
  => no transposes anywhere in the attention.

All matmuls take fp16 inputs (1 PE cycle/row vs 4 for fp32) and accumulate in
fp32 PSUM. Host-side numpy emulation: rel_l2 error vs fp32 reference ~ 3.6e-5.
"""

import sys

sys.path.insert(0, "/opt/trn_rl_repo")

import numpy as np

import concourse.bass as bass
from concourse import bacc
import concourse.tile as tile
from concourse import bass_isa, mybir
from concourse.bass import ds
from concourse.bass_utils import run_bass_kernel_spmd

P = 128
C = 512
CT = C // P          # 4 channel tiles
N = 4096
NCH = N // 512       # 8 column chunks of 512
NT = N // P          # 32 position tiles of 128
G = 32               # groups
GS = C // G          # 16 channels per group
EPS = 1e-5
SCALE = float(C) ** -0.5

F16 = mybir.dt.float16
F32 = mybir.dt.float32

TUNE = dict(
    work_bufs=2,
    acc_bufs=4,
    s_bufs=3,
    es_bufs=4,
    xch_bufs=2,
    asb_bufs=5,
    ot_bufs=3,
    pv_depth=2,
    # timing-bisection variants (leave default for the real kernel)
    v_skip_attn=False,    # stop after QKV
    v_exp_copy=False,     # exp -> Copy (no Exp table)
    v_skip_dacc=False,    # no DVE denominator accumulation
    v_skip_pred=False,    # no gpsimd partition_all_reduce
    v_no_xdma=False,      # no x streaming (reuse one garbage chunk)
)


def build_nc(with_cb: bool, tune=None):
    t_ = dict(TUNE)
    if tune:
        t_.update(tune)
    nc = bacc.Bacc(trn_type="TRN2")

    x_d = nc.dram_tensor("x", [C, N], F32, kind="ExternalInput")
    wq_d = nc.dram_tensor("wqT", [C, C], F16, kind="ExternalInput")
    wk_d = nc.dram_tensor("wkT", [C, C], F16, kind="ExternalInput")
    wv_d = nc.dram_tensor("wvT", [C, C], F16, kind="ExternalInput")
    wp_d = nc.dram_tensor("wpT", [C, C], F16, kind="ExternalInput")
    bqk_d = nc.dram_tensor("bqk", [P, 2, CT], F32, kind="ExternalInput")
    gnw_d = nc.dram_tensor("gnw", [P, CT, 2], F32, kind="ExternalInput")
    gstat_d = nc.dram_tensor("gstat", [P, CT, G], F32, kind="ExternalInput")
    gexp_d = nc.dram_tensor("gexp", [P, CT, P], F32, kind="ExternalInput")
    if with_cb:
        cb_d = nc.dram_tensor("cb", [P, CT], F32, kind="ExternalInput")
    out_d = nc.dram_tensor("out", [C, N], F32, kind="ExternalOutput")

    x3 = x_d.rearrange("(t p) n -> p t n", p=P)
    o3 = out_d.rearrange("(t p) n -> p t n", p=P)
    w3 = {
        "q": wq_d.rearrange("(t p) o -> p t o", p=P),
        "k": wk_d.rearrange("(t p) o -> p t o", p=P),
        "v": wv_d.rearrange("(t p) o -> p t o", p=P),
        "p": wp_d.rearrange("(t p) o -> p t o", p=P),
    }

    from contextlib import ExitStack

    with tile.TileContext(nc) as tc, ExitStack() as ctx:
        persist = ctx.enter_context(tc.tile_pool(name="persist", bufs=1))
        work = ctx.enter_context(tc.tile_pool(name="work", bufs=t_["work_bufs"]))
        ps_acc = ctx.enter_context(
            tc.tile_pool(name="ps_acc", bufs=t_["acc_bufs"], space="PSUM")
        )
        ps_s = ctx.enter_context(
            tc.tile_pool(name="ps_s", bufs=t_["s_bufs"], space="PSUM")
        )
        ps_gn = ctx.enter_context(tc.tile_pool(name="ps_gn", bufs=1, space="PSUM"))

        # ---------------- persistent weights / constants ----------------
        wsb = {}
        for kk in ("q", "k", "v", "p"):
            wsb[kk] = persist.tile([P, CT, C], F16, tag=f"w{kk}", name=f"w{kk}_sb")
            nc.sync.dma_start(out=wsb[kk], in_=w3[kk])
        bqk_sb = persist.tile([P, 2, CT], F32, tag="bqk")
        nc.sync.dma_start(out=bqk_sb, in_=bqk_d.ap())
        gnw_sb = persist.tile([P, CT, 2], F32, tag="gnw")
        nc.sync.dma_start(out=gnw_sb, in_=gnw_d.ap())
        gstat_sb = persist.tile([P, CT, G], F32, tag="gstat")
        nc.sync.dma_start(out=gstat_sb, in_=gstat_d.ap())
        gexp_sb = persist.tile([P, CT, P], F32, tag="gexp")
        nc.sync.dma_start(out=gexp_sb, in_=gexp_d.ap())
        if with_cb:
            cb_sb = persist.tile([P, CT], F32, tag="cb")
            nc.sync.dma_start(out=cb_sb, in_=cb_d.ap())
        eps_sb = persist.tile([G, 1], F32, tag="eps")
        nc.vector.memset(eps_sb, EPS)
        ones_col = persist.tile([P, 1], F32, tag="ones_col")
        nc.vector.memset(ones_col, 1.0)
        ones_row = persist.tile([1, P], F32, tag="ones_row")
        nc.vector.memset(ones_row, 1.0)

        Q_sb = persist.tile([P, CT, N], F16, tag="Q")
        K_sb = persist.tile([P, CT, N], F16, tag="K")
        V_sb = persist.tile([P, NT, C], F16, tag="V")

        SD = nc.vector.BN_STATS_DIM
        AD = nc.vector.BN_AGGR_DIM

        # ---------------- phase 1: GN statistics (stream x) ----------------
        stats = persist.tile([P, CT, NCH, SD], F32, tag="stats")
        for nch in range(NCH):
            xch = work.tile(
                [P, CT, 512], F32, tag="xch", bufs=t_["xch_bufs"], name="xch"
            )
            if not t_["v_no_xdma"]:
                nc.sync.dma_start(out=xch, in_=x3[:, :, ds(nch * 512, 512)])
            for t in range(CT):
                nc.vector.bn_stats(out=stats[:, t, nch, :], in_=xch[:, t, :])
        mv = persist.tile([P, CT, AD], F32, tag="mv")
        rs = persist.tile([P, CT, 2], F32, tag="rs")
        for t in range(CT):
            nc.vector.bn_aggr(out=mv[:, t, :], in_=stats[:, t, :, :])
            # rs = [mean, E[x^2]] per channel
            nc.vector.tensor_mul(rs[:, t, 1:2], mv[:, t, 0:1], mv[:, t, 0:1])
            nc.vector.tensor_add(rs[:, t, 1:2], rs[:, t, 1:2], mv[:, t, 1:2])
            nc.vector.tensor_copy(rs[:, t, 0:1], mv[:, t, 0:1])

        # group reduce: pstat[g, :] = [mu_g, E2_g]  (1/GS folded into gstat)
        pstat = ps_gn.tile([G, 2], F32, tag="psgn", name="pstat")
        for t in range(CT):
            nc.tensor.matmul(
                pstat,
                lhsT=gstat_sb[:, t, :],
                rhs=rs[:, t, :],
                start=(t == 0),
                stop=(t == CT - 1),
            )
        gmr = persist.tile([P, 2], F32, tag="gmr")  # [mu_g, rstd_g], zero padded
        nc.vector.memset(gmr, 0.0)
        gst = persist.tile([G, 2], F32, tag="gst")  # pstat evacuated to SBUF
        nc.vector.tensor_copy(gst, pstat)
        gvar = persist.tile([G, 1], F32, tag="gvar")
        nc.vector.tensor_copy(gmr[:G, 0:1], gst[:, 0:1])
        nc.vector.tensor_mul(gvar, gst[:, 0:1], gst[:, 0:1])
        nc.vector.tensor_tensor(
            gvar, gst[:, 1:2], gvar, op=mybir.AluOpType.subtract
        )
        nc.scalar.activation(
            gmr[:G, 1:2], gvar, mybir.ActivationFunctionType.Sqrt, bias=eps_sb
        )
        nc.vector.reciprocal(gmr[:G, 1:2], gmr[:G, 1:2])

        # expand to channels, then per-channel affine A,B:
        # h = A*x + B with A = gamma*rstd, B = beta - mu*A
        AB = persist.tile([P, CT, 2], F32, tag="AB")
        for t in range(CT):
            pexp = ps_gn.tile([P, 2], F32, tag="psgn", name="pexp")
            nc.tensor.matmul(
                pexp, lhsT=gexp_sb[:, t, :], rhs=gmr, start=True, stop=True
            )
            nc.vector.tensor_mul(AB[:, t, 0:1], gnw_sb[:, t, 0:1], pexp[:, 1:2])
            nc.vector.tensor_mul(AB[:, t, 1:2], pexp[:, 0:1], AB[:, t, 0:1])
            nc.vector.tensor_tensor(
                AB[:, t, 1:2],
                gnw_sb[:, t, 1:2],
                AB[:, t, 1:2],
                op=mybir.AluOpType.subtract,
            )

        # ------------- phase 2+3: normalize + QKV, per 512-col chunk -------------
        for nch in range(NCH):
            xch = work.tile(
                [P, CT, 512], F32, tag="xch", bufs=t_["xch_bufs"], name="xch"
            )
            if not t_["v_no_xdma"]:
                nc.sync.dma_start(out=xch, in_=x3[:, :, ds(nch * 512, 512)])
            hch = work.tile([P, CT, 512], F16, tag="hch", name="hch")
            for t in range(CT):
                nc.vector.tensor_scalar(
                    out=hch[:, t, :],
                    in0=xch[:, t, :],
                    scalar1=AB[:, t, 0:1],
                    scalar2=AB[:, t, 1:2],
                    op0=mybir.AluOpType.mult,
                    op1=mybir.AluOpType.add,
                )
            for bcol, (kk, dst) in enumerate((("q", Q_sb), ("k", K_sb))):
                for o in range(CT):
                    pq = ps_acc.tile([P, 512], F32, tag="acc", name="pq")
                    for t in range(CT):
                        nc.tensor.matmul(
                            pq,
                            lhsT=wsb[kk][:, t, ds(o * P, P)],
                            rhs=hch[:, t, :],
                            start=(t == 0),
                            stop=(t == CT - 1),
                        )
                    nc.scalar.activation(
                        dst[:, o, ds(nch * 512, 512)],
                        pq,
                        mybir.ActivationFunctionType.Identity,
                        bias=bqk_sb[:, bcol, o : o + 1],
                    )
            for j in range(4):
                m = nch * 4 + j
                pv = ps_acc.tile([P, 512], F32, tag="acc", name="pv")
                for t in range(CT):
                    nc.tensor.matmul(
                        pv,
                        lhsT=hch[:, t, ds(j * P, P)],
                        rhs=wsb["v"][:, t, :],
                        start=(t == 0),
                        stop=(t == CT - 1),
                    )
                nc.scalar.copy(V_sb[:, m, :], pv)

        # ------------- phase 4: attention + projection, per query block -------------
        if t_["v_skip_attn"]:
            for o in range(CT):
                vo = work.tile([P, N], F32, tag="vout", bufs=2, name="vo")
                nc.vector.tensor_copy(vo, V_sb[:, o * 8 : (o + 1) * 8, :])
                nc.sync.dma_start(out=o3[:, o, :], in_=vo)
        blocks = [] if t_["v_skip_attn"] else list(range(NCH))
        for nb in blocks:
            dacc = work.tile([P, 512], F32, tag="dacc", name="dacc")
            nc.vector.memset(dacc, 0.0)
            pa = [
                ps_acc.tile([P, 512], F32, tag="acc", name=f"pa{c}")
                for c in range(CT)
            ]
            DEPTH = t_["pv_depth"]  # S->exp->PV software pipeline depth
            es_tiles = [None] * NT

            def pv_group(mm, last):
                esp = es_tiles[mm]
                for c in range(CT):
                    nc.tensor.matmul(
                        pa[c],
                        lhsT=V_sb[:, mm, ds(c * P, P)],
                        rhs=esp,
                        start=(mm == 0),
                        stop=last,
                    )

            for m in range(NT):
                # scores^T tile: S[m-tile, nb chunk]
                ps = ps_s.tile([P, 512], F32, tag="s", name="ps")
                for t in range(CT):
                    nc.tensor.matmul(
                        ps,
                        lhsT=K_sb[:, t, ds(m * P, P)],
                        rhs=Q_sb[:, t, ds(nb * 512, 512)],
                        start=(t == 0),
                        stop=(t == CT - 1),
                    )
                # software pipeline: PV for m-DEPTH queued before exp(m) so the
                # PE never waits on the ACT engine's exp
                if m >= DEPTH:
                    pv_group(m - DEPTH, last=False)
                es = work.tile([P, 512], F16, tag="es", bufs=t_["es_bufs"], name="es")
                nc.scalar.activation(
                    es,
                    ps,
                    mybir.ActivationFunctionType.Copy
                    if t_["v_exp_copy"]
                    else mybir.ActivationFunctionType.Exp,
                    scale=SCALE,
                )
                if not t_["v_skip_dacc"]:
                    nc.vector.tensor_add(dacc, dacc, es)
                es_tiles[m] = es
            for mm in range(NT - DEPTH, NT):
                pv_group(mm, last=(mm == NT - 1))

            rd = work.tile([P, 512], F32, tag="rd", name="rd")
            if t_["v_skip_pred"]:
                nc.vector.reciprocal(rd, dacc)
            else:
                # D = ones^T @ dacc on the PE (one fp32 mm), reciprocal on one
                # partition, broadcast back via a K=1 outer-product mm.
                psd = ps_gn.tile([1, 512], F32, tag="psgn", name="psd")
                nc.tensor.matmul(
                    psd, lhsT=ones_col, rhs=dacc, start=True, stop=True
                )
                rdrow = work.tile([1, 512], F32, tag="rdrow", name="rdrow")
                nc.vector.reciprocal(rdrow, psd)
                psrd = ps_s.tile([P, 512], F32, tag="s", name="psrd")
                nc.tensor.matmul(
                    psrd, lhsT=ones_row, rhs=rdrow, start=True, stop=True
                )
                nc.scalar.copy(rd, psrd)

            asb = []
            for c in range(CT):
                a = work.tile(
                    [P, 512], F16, tag="asb", bufs=t_["asb_bufs"], name=f"asb{c}"
                )
                nc.scalar.copy(a, pa[c])
                asb.append(a)

            xch = work.tile(
                [P, CT, 512], F32, tag="xch", bufs=t_["xch_bufs"], name="xch"
            )
            if not t_["v_no_xdma"]:
                nc.sync.dma_start(out=xch, in_=x3[:, :, ds(nb * 512, 512)])
            for o in range(CT):
                po = ps_acc.tile([P, 512], F32, tag="acc", name="po")
                for c in range(CT):
                    nc.tensor.matmul(
                        po,
                        lhsT=wsb["p"][:, c, ds(o * P, P)],
                        rhs=asb[c],
                        start=(c == 0),
                        stop=(c == CT - 1),
                    )
                ot = work.tile([P, 512], F32, tag="ot", bufs=t_["ot_bufs"], name="ot")
                nc.vector.tensor_mul(ot, po, rd)
                if with_cb:
                    nc.vector.tensor_scalar(
                        out=ot,
                        in0=ot,
                        scalar1=cb_sb[:, o : o + 1],
                        op0=mybir.AluOpType.add,
                    )
                nc.vector.tensor_add(ot, ot, xch[:, o, :])
                nc.sync.dma_start(out=o3[:, o, ds(nb * 512, 512)], in_=ot)

    nc.compile()
    return nc


_NC_CACHE = {}


def get_nc(with_cb: bool, tune=None):
    key = (with_cb, tuple(sorted((tune or {}).items())))
    if key not in _NC_CACHE:
        _NC_CACHE[key] = build_nc(with_cb, tune)
    return _NC_CACHE[key]


def make_in_maps(x, gn_gamma, gn_beta, wq, bq, wk, bk, wv, bv, wp, bp):
    """Host-side prep: returns (in_maps list for 8 cores, with_cb flag)."""
    x = np.asarray(x, dtype=np.float32)
    B = x.shape[0]
    assert x.shape == (8, C, 64, 64)

    f32 = np.float32
    wqT = np.ascontiguousarray(np.asarray(wq, f32).T).astype(np.float16)
    wkT = np.ascontiguousarray(np.asarray(wk, f32).T).astype(np.float16)
    wvT = np.ascontiguousarray(np.asarray(wv, f32).T).astype(np.float16)
    wpT = np.ascontiguousarray(np.asarray(wp, f32).T).astype(np.float16)

    bq = np.asarray(bq, f32)
    bk = np.asarray(bk, f32)
    bqk = np.ascontiguousarray(
        np.stack([bq.reshape(CT, P).T, bk.reshape(CT, P).T], axis=1)
    )  # [P, 2, CT]
    gnw = np.ascontiguousarray(
        np.stack(
            [np.asarray(gn_gamma, f32).reshape(CT, P).T,
             np.asarray(gn_beta, f32).reshape(CT, P).T],
            axis=2,
        )
    )  # [P, CT, 2]

    gstat = np.zeros((P, CT, G), f32)
    for t in range(CT):
        for p in range(P):
            gstat[p, t, 8 * t + p // GS] = 1.0 / GS
    gexp = np.zeros((P, CT, P), f32)  # [g(padded to 128), t, c]
    for t in range(CT):
        for c in range(P):
            gexp[8 * t + c // GS, t, c] = 1.0

    cb = (np.asarray(wp, f32) @ np.asarray(bv, f32) + np.asarray(bp, f32)).astype(f32)
    with_cb = bool(np.abs(cb).max() > 0)
    cb4 = np.ascontiguousarray(cb.reshape(CT, P).T)  # [P, CT]

    shared = {
        "wqT": wqT, "wkT": wkT, "wvT": wvT, "wpT": wpT,
        "bqk": bqk, "gnw": gnw, "gstat": gstat, "gexp": gexp,
    }
    if with_cb:
        shared["cb"] = cb4

    in_maps = []
    for b in range(B):
        m = dict(shared)
        m["x"] = np.ascontiguousarray(x[b].reshape(C, N))
        in_maps.append(m)
    return in_maps, with_cb


def kernel(x, gn_gamma, gn_beta, wq, bq, wk, bk, wv, bv, wp, bp):
    in_maps, with_cb = make_in_maps(
        x, gn_gamma, gn_beta, wq, bq, wk, bk, wv, bv, wp, bp
    )
    nc = get_nc(with_cb)
    res = run_bass_kernel_spmd(nc, in_maps, core_ids=list(range(8)))
    outs = [res.results[b]["out"].reshape(C, 64, 64) for b in range(8)]
    return np.stack(outs).astype(np.float32)


# revision 12
# speedup vs baseline: 3.2185x; 1.8043x over previous
"""AttnBlock (GroupNorm + 1x1-conv self-attention + residual) on 8 trn2 cores.

Sharding: pure data parallel -- batch b runs on core b (B=8, n_cores=8).
Each core computes its whole batch: GN -> QKV -> attention -> proj -> residual.

Layout choices (per core, C=512, N=H*W=4096):
  h, Q, K stored [C, N] (channels on partitions, 4 tiles of 128)
  V stored [N, C] (positions on partitions, 32 tiles of 128)
  S^T = K^T Q computed [keys m, queries n]; softmax over m = partition dim
  across 32 m-tiles: denominator via DVE accumulation + gpsimd
  partition_all_reduce; division deferred to after the output projection
  (out = x + (wp @ A_unnorm) / D).
  => no transposes anywhere in the attention.

All matmuls take fp16 inputs (1 PE cycle/row vs 4 for fp32) and accumulate in
fp32 PSUM. Host-side numpy emulation: rel_l2 error vs fp32 reference ~ 3.6e-5.
"""

import sys

sys.path.insert(0, "/opt/trn_rl_repo")

import numpy as np

import concourse.bass as bass
from concourse import bacc
import concourse.tile as tile
from concourse import bass_isa, mybir
from concourse.bass import ds
from concourse.bass_utils import run_bass_kernel_spmd

P = 128
C = 512
CT = C // P          # 4 channel tiles
N = 4096
NCH = N // 512       # 8 column chunks of 512
NT = N // P          # 32 position tiles of 128
G = 32               # groups
GS = C // G          # 16 channels per group
EPS = 1e-5
SCALE = float(C) ** -0.5

F16 = mybir.dt.float16
F32 = mybir.dt.float32
F8 = mybir.dt.float8e4
LN16 = 2.772588722239781  # ln(16): expS stored as exp(s)/16 in fp8 (cancels in A/D)

TUNE = dict(
    work_bufs=2,
    acc_bufs=4,
    s_bufs=3,
    es_bufs=4,
    xch_bufs=2,
    asb_bufs=5,
    ot_bufs=3,
    pv_depth=2,
    fp8_s=False,     # scores matmul in fp8e4m3 + DoubleRow (Q,K stored fp8)
    fp8_pv=False,    # PV matmul in fp8e4m3 + DoubleRow (V, expS stored fp8)
    # timing-bisection variants (leave default for the real kernel)
    v_skip_attn=False,    # stop after QKV
    v_exp_copy=False,     # exp -> Copy (no Exp table)
    v_skip_dacc=False,    # no DVE denominator accumulation
    v_skip_pred=False,    # no gpsimd partition_all_reduce
    v_no_xdma=False,      # no x streaming (reuse one garbage chunk)
)


def build_nc(with_cb: bool, tune=None):
    t_ = dict(TUNE)
    if tune:
        t_.update(tune)
    nc = bacc.Bacc(trn_type="TRN2")

    x_d = nc.dram_tensor("x", [C, N], F32, kind="ExternalInput")
    wq_d = nc.dram_tensor("wqT", [C, C], F16, kind="ExternalInput")
    wk_d = nc.dram_tensor("wkT", [C, C], F16, kind="ExternalInput")
    wv_d = nc.dram_tensor("wvT", [C, C], F16, kind="ExternalInput")
    wp_d = nc.dram_tensor("wpT", [C, C], F16, kind="ExternalInput")
    bqk_d = nc.dram_tensor("bqk", [P, 2, CT], F32, kind="ExternalInput")
    gnw_d = nc.dram_tensor("gnw", [P, CT, 2], F32, kind="ExternalInput")
    gstat_d = nc.dram_tensor("gstat", [P, CT, G], F32, kind="ExternalInput")
    gexp_d = nc.dram_tensor("gexp", [P, CT, P], F32, kind="ExternalInput")
    if with_cb:
        cb_d = nc.dram_tensor("cb", [P, CT], F32, kind="ExternalInput")
    out_d = nc.dram_tensor("out", [C, N], F32, kind="ExternalOutput")

    x3 = x_d.rearrange("(t p) n -> p t n", p=P)
    o3 = out_d.rearrange("(t p) n -> p t n", p=P)
    w3 = {
        "q": wq_d.rearrange("(t p) o -> p t o", p=P),
        "k": wk_d.rearrange("(t p) o -> p t o", p=P),
        "v": wv_d.rearrange("(t p) o -> p t o", p=P),
        "p": wp_d.rearrange("(t p) o -> p t o", p=P),
    }

    from contextlib import ExitStack

    with tile.TileContext(nc) as tc, ExitStack() as ctx:
        persist = ctx.enter_context(tc.tile_pool(name="persist", bufs=1))
        work = ctx.enter_context(tc.tile_pool(name="work", bufs=t_["work_bufs"]))
        ps_acc = ctx.enter_context(
            tc.tile_pool(name="ps_acc", bufs=t_["acc_bufs"], space="PSUM")
        )
        ps_s = ctx.enter_context(
            tc.tile_pool(name="ps_s", bufs=t_["s_bufs"], space="PSUM")
        )
        ps_gn = ctx.enter_context(tc.tile_pool(name="ps_gn", bufs=1, space="PSUM"))

        # ---------------- persistent weights / constants ----------------
        wsb = {}
        for kk in ("q", "k", "v", "p"):
            wsb[kk] = persist.tile([P, CT, C], F16, tag=f"w{kk}", name=f"w{kk}_sb")
            nc.sync.dma_start(out=wsb[kk], in_=w3[kk])
        bqk_sb = persist.tile([P, 2, CT], F32, tag="bqk")
        nc.sync.dma_start(out=bqk_sb, in_=bqk_d.ap())
        gnw_sb = persist.tile([P, CT, 2], F32, tag="gnw")
        nc.sync.dma_start(out=gnw_sb, in_=gnw_d.ap())
        gstat_sb = persist.tile([P, CT, G], F32, tag="gstat")
        nc.sync.dma_start(out=gstat_sb, in_=gstat_d.ap())
        gexp_sb = persist.tile([P, CT, P], F32, tag="gexp")
        nc.sync.dma_start(out=gexp_sb, in_=gexp_d.ap())
        if with_cb:
            cb_sb = persist.tile([P, CT], F32, tag="cb")
            nc.sync.dma_start(out=cb_sb, in_=cb_d.ap())
        eps_sb = persist.tile([G, 1], F32, tag="eps")
        nc.vector.memset(eps_sb, EPS)
        ones_col = persist.tile([P, 1], F32, tag="ones_col")
        nc.vector.memset(ones_col, 1.0)
        ones_row = persist.tile([1, P], F32, tag="ones_row")
        nc.vector.memset(ones_row, 1.0)

        QK_DT = F8 if t_["fp8_s"] else F16
        V_DT = F8 if t_["fp8_pv"] else F16
        Q_sb = persist.tile([P, CT, N], QK_DT, tag="Q")
        K_sb = persist.tile([P, CT, N], QK_DT, tag="K")
        V_sb = persist.tile([P, NT, C], V_DT, tag="V")
        if t_["fp8_pv"]:
            ones8 = persist.tile([P, 2, 1], F8, tag="ones8")
            nc.vector.memset(ones8, 1.0)
            mln16 = persist.tile([P, 1], F32, tag="mln16")
            nc.vector.memset(mln16, -LN16)

        SD = nc.vector.BN_STATS_DIM
        AD = nc.vector.BN_AGGR_DIM

        # ---------------- phase 1: GN statistics (stream x) ----------------
        stats = persist.tile([P, CT, NCH, SD], F32, tag="stats")
        for nch in range(NCH):
            xch = work.tile(
                [P, CT, 512], F32, tag="xch", bufs=t_["xch_bufs"], name="xch"
            )
            if not t_["v_no_xdma"]:
                nc.sync.dma_start(out=xch, in_=x3[:, :, ds(nch * 512, 512)])
            for t in range(CT):
                nc.vector.bn_stats(out=stats[:, t, nch, :], in_=xch[:, t, :])
        mv = persist.tile([P, CT, AD], F32, tag="mv")
        rs = persist.tile([P, CT, 2], F32, tag="rs")
        for t in range(CT):
            nc.vector.bn_aggr(out=mv[:, t, :], in_=stats[:, t, :, :])
            # rs = [mean, E[x^2]] per channel
            nc.vector.tensor_mul(rs[:, t, 1:2], mv[:, t, 0:1], mv[:, t, 0:1])
            nc.vector.tensor_add(rs[:, t, 1:2], rs[:, t, 1:2], mv[:, t, 1:2])
            nc.vector.tensor_copy(rs[:, t, 0:1], mv[:, t, 0:1])

        # group reduce: pstat[g, :] = [mu_g, E2_g]  (1/GS folded into gstat)
        pstat = ps_gn.tile([G, 2], F32, tag="psgn", name="pstat")
        for t in range(CT):
            nc.tensor.matmul(
                pstat,
                lhsT=gstat_sb[:, t, :],
                rhs=rs[:, t, :],
                start=(t == 0),
                stop=(t == CT - 1),
            )
        gmr = persist.tile([P, 2], F32, tag="gmr")  # [mu_g, rstd_g], zero padded
        nc.vector.memset(gmr, 0.0)
        gst = persist.tile([G, 2], F32, tag="gst")  # pstat evacuated to SBUF
        nc.vector.tensor_copy(gst, pstat)
        gvar = persist.tile([G, 1], F32, tag="gvar")
        nc.vector.tensor_copy(gmr[:G, 0:1], gst[:, 0:1])
        nc.vector.tensor_mul(gvar, gst[:, 0:1], gst[:, 0:1])
        nc.vector.tensor_tensor(
            gvar, gst[:, 1:2], gvar, op=mybir.AluOpType.subtract
        )
        nc.scalar.activation(
            gmr[:G, 1:2], gvar, mybir.ActivationFunctionType.Sqrt, bias=eps_sb
        )
        nc.vector.reciprocal(gmr[:G, 1:2], gmr[:G, 1:2])

        # expand to channels, then per-channel affine A,B:
        # h = A*x + B with A = gamma*rstd, B = beta - mu*A
        AB = persist.tile([P, CT, 2], F32, tag="AB")
        for t in range(CT):
            pexp = ps_gn.tile([P, 2], F32, tag="psgn", name="pexp")
            nc.tensor.matmul(
                pexp, lhsT=gexp_sb[:, t, :], rhs=gmr, start=True, stop=True
            )
            nc.vector.tensor_mul(AB[:, t, 0:1], gnw_sb[:, t, 0:1], pexp[:, 1:2])
            nc.vector.tensor_mul(AB[:, t, 1:2], pexp[:, 0:1], AB[:, t, 0:1])
            nc.vector.tensor_tensor(
                AB[:, t, 1:2],
                gnw_sb[:, t, 1:2],
                AB[:, t, 1:2],
                op=mybir.AluOpType.subtract,
            )

        # ------------- phase 2+3: normalize + QKV, per 512-col chunk -------------
        for nch in range(NCH):
            xch = work.tile(
                [P, CT, 512], F32, tag="xch", bufs=t_["xch_bufs"], name="xch"
            )
            if not t_["v_no_xdma"]:
                nc.sync.dma_start(out=xch, in_=x3[:, :, ds(nch * 512, 512)])
            hch = work.tile([P, CT, 512], F16, tag="hch", name="hch")
            for t in range(CT):
                nc.vector.tensor_scalar(
                    out=hch[:, t, :],
                    in0=xch[:, t, :],
                    scalar1=AB[:, t, 0:1],
                    scalar2=AB[:, t, 1:2],
                    op0=mybir.AluOpType.mult,
                    op1=mybir.AluOpType.add,
                )
            for bcol, (kk, dst) in enumerate((("q", Q_sb), ("k", K_sb))):
                for o in range(CT):
                    pq = ps_acc.tile([P, 512], F32, tag="acc", name="pq")
                    for t in range(CT):
                        nc.tensor.matmul(
                            pq,
                            lhsT=wsb[kk][:, t, ds(o * P, P)],
                            rhs=hch[:, t, :],
                            start=(t == 0),
                            stop=(t == CT - 1),
                        )
                    nc.scalar.activation(
                        dst[:, o, ds(nch * 512, 512)],
                        pq,
                        mybir.ActivationFunctionType.Identity,
                        bias=bqk_sb[:, bcol, o : o + 1],
                    )
            for j in range(4):
                m = nch * 4 + j
                pv = ps_acc.tile([P, 512], F32, tag="acc", name="pv")
                for t in range(CT):
                    nc.tensor.matmul(
                        pv,
                        lhsT=hch[:, t, ds(j * P, P)],
                        rhs=wsb["v"][:, t, :],
                        start=(t == 0),
                        stop=(t == CT - 1),
                    )
                nc.scalar.copy(V_sb[:, m, :], pv)

        # ------------- phase 4: attention + projection, per query block -------------
        if t_["v_skip_attn"]:
            for o in range(CT):
                vo = work.tile([P, N], F32, tag="vout", bufs=2, name="vo")
                nc.vector.tensor_copy(vo, V_sb[:, o * 8 : (o + 1) * 8, :])
                nc.sync.dma_start(out=o3[:, o, :], in_=vo)
        blocks = [] if t_["v_skip_attn"] else list(range(NCH))
        DR = mybir.MatmulPerfMode.DoubleRow
        exp_func = (
            mybir.ActivationFunctionType.Copy
            if t_["v_exp_copy"]
            else mybir.ActivationFunctionType.Exp
        )
        for nb in blocks:
            pa = [
                ps_acc.tile([P, 512], F32, tag="acc", name=f"pa{c}")
                for c in range(CT)
            ]

            def s_group(m, ps):
                if t_["fp8_s"]:
                    for tp in (0, 2):
                        nc.tensor.matmul(
                            ps,
                            lhsT=K_sb[:, tp : tp + 2, ds(m * P, P)],
                            rhs=Q_sb[:, tp : tp + 2, ds(nb * 512, 512)],
                            start=(tp == 0),
                            stop=(tp == 2),
                            perf_mode=DR,
                        )
                else:
                    for t in range(CT):
                        nc.tensor.matmul(
                            ps,
                            lhsT=K_sb[:, t, ds(m * P, P)],
                            rhs=Q_sb[:, t, ds(nb * 512, 512)],
                            start=(t == 0),
                            stop=(t == CT - 1),
                        )

            if t_["fp8_pv"]:
                # expS stored as fp8 pairs [P, 2, 512]; PV and the softmax
                # denominator both via DoubleRow matmuls (D on PE, not DVE)
                NP = NT // 2
                PDP = max(1, t_["pv_depth"] // 2)  # pipeline depth in pairs
                es_pairs = [None] * NP
                psd = ps_gn.tile([1, 512], F32, tag="psgn", name="psd")

                def pv_group8(j, last):
                    es2 = es_pairs[j]
                    for c in range(CT):
                        nc.tensor.matmul(
                            pa[c],
                            lhsT=V_sb[:, 2 * j : 2 * j + 2, ds(c * P, P)],
                            rhs=es2,
                            start=(j == 0),
                            stop=last,
                            perf_mode=DR,
                        )
                    nc.tensor.matmul(
                        psd,
                        lhsT=ones8,
                        rhs=es2,
                        start=(j == 0),
                        stop=last,
                        perf_mode=DR,
                    )

                for m in range(NT):
                    ps = ps_s.tile([P, 512], F32, tag="s", name="ps")
                    s_group(m, ps)
                    j = m // 2
                    if m % 2 == 1 and j - PDP >= 0:
                        pv_group8(j - PDP, last=False)
                    if m % 2 == 0:
                        es_pairs[j] = work.tile(
                            [P, 2, 512], F8, tag="es", bufs=t_["es_bufs"], name="es"
                        )
                    nc.scalar.activation(
                        es_pairs[j][:, m % 2, :],
                        ps,
                        exp_func,
                        scale=SCALE,
                        bias=mln16,
                    )
                for j in range(NP - PDP, NP):
                    pv_group8(j, last=(j == NP - 1))
                rdrow = work.tile([1, 512], F32, tag="rdrow", name="rdrow")
                nc.vector.reciprocal(rdrow, psd)
            else:
                dacc = work.tile([P, 512], F32, tag="dacc", name="dacc")
                nc.vector.memset(dacc, 0.0)
                DEPTH = t_["pv_depth"]  # S->exp->PV software pipeline depth
                es_tiles = [None] * NT

                def pv_group(mm, last):
                    esp = es_tiles[mm]
                    for c in range(CT):
                        nc.tensor.matmul(
                            pa[c],
                            lhsT=V_sb[:, mm, ds(c * P, P)],
                            rhs=esp,
                            start=(mm == 0),
                            stop=last,
                        )

                for m in range(NT):
                    # scores^T tile: S[m-tile, nb chunk]
                    ps = ps_s.tile([P, 512], F32, tag="s", name="ps")
                    s_group(m, ps)
                    # software pipeline: PV for m-DEPTH queued before exp(m) so
                    # the PE never waits on the ACT engine's exp
                    if m >= DEPTH:
                        pv_group(m - DEPTH, last=False)
                    es = work.tile(
                        [P, 512], F16, tag="es", bufs=t_["es_bufs"], name="es"
                    )
                    nc.scalar.activation(es, ps, exp_func, scale=SCALE)
                    if not t_["v_skip_dacc"]:
                        nc.vector.tensor_add(dacc, dacc, es)
                    es_tiles[m] = es
                for mm in range(NT - DEPTH, NT):
                    pv_group(mm, last=(mm == NT - 1))

            rd = work.tile([P, 512], F32, tag="rd", name="rd")
            if t_["fp8_pv"]:
                psrd = ps_s.tile([P, 512], F32, tag="s", name="psrd")
                nc.tensor.matmul(
                    psrd, lhsT=ones_row, rhs=rdrow, start=True, stop=True
                )
                nc.scalar.copy(rd, psrd)
            elif t_["v_skip_pred"]:
                nc.vector.reciprocal(rd, dacc)
            else:
                # D = ones^T @ dacc on the PE (one fp32 mm), reciprocal on one
                # partition, broadcast back via a K=1 outer-product mm.
                psd = ps_gn.tile([1, 512], F32, tag="psgn", name="psd")
                nc.tensor.matmul(
                    psd, lhsT=ones_col, rhs=dacc, start=True, stop=True
                )
                rdrow = work.tile([1, 512], F32, tag="rdrow", name="rdrow")
                nc.vector.reciprocal(rdrow, psd)
                psrd = ps_s.tile([P, 512], F32, tag="s", name="psrd")
                nc.tensor.matmul(
                    psrd, lhsT=ones_row, rhs=rdrow, start=True, stop=True
                )
                nc.scalar.copy(rd, psrd)

            asb = []
            for c in range(CT):
                a = work.tile(
                    [P, 512], F16, tag="asb", bufs=t_["asb_bufs"], name=f"asb{c}"
                )
                nc.scalar.copy(a, pa[c])
                asb.append(a)

            xch = work.tile(
                [P, CT, 512], F32, tag="xch", bufs=t_["xch_bufs"], name="xch"
            )
            if not t_["v_no_xdma"]:
                nc.sync.dma_start(out=xch, in_=x3[:, :, ds(nb * 512, 512)])
            for o in range(CT):
                po = ps_acc.tile([P, 512], F32, tag="acc", name="po")
                for c in range(CT):
                    nc.tensor.matmul(
                        po,
                        lhsT=wsb["p"][:, c, ds(o * P, P)],
                        rhs=asb[c],
                        start=(c == 0),
                        stop=(c == CT - 1),
                    )
                ot = work.tile([P, 512], F32, tag="ot", bufs=t_["ot_bufs"], name="ot")
                nc.vector.tensor_mul(ot, po, rd)
                if with_cb:
                    nc.vector.tensor_scalar(
                        out=ot,
                        in0=ot,
                        scalar1=cb_sb[:, o : o + 1],
                        op0=mybir.AluOpType.add,
                    )
                nc.vector.tensor_add(ot, ot, xch[:, o, :])
                nc.sync.dma_start(out=o3[:, o, ds(nb * 512, 512)], in_=ot)

    nc.compile()
    return nc


_NC_CACHE = {}


def get_nc(with_cb: bool, tune=None):
    key = (with_cb, tuple(sorted((tune or {}).items())))
    if key not in _NC_CACHE:
        _NC_CACHE[key] = build_nc(with_cb, tune)
    return _NC_CACHE[key]


def make_in_maps(x, gn_gamma, gn_beta, wq, bq, wk, bk, wv, bv, wp, bp):
    """Host-side prep: returns (in_maps list for 8 cores, with_cb flag)."""
    x = np.asarray(x, dtype=np.float32)
    B = x.shape[0]
    assert x.shape == (8, C, 64, 64)

    f32 = np.float32
    wqT = np.ascontiguousarray(np.asarray(wq, f32).T).astype(np.float16)
    wkT = np.ascontiguousarray(np.asarray(wk, f32).T).astype(np.float16)
    wvT = np.ascontiguousarray(np.asarray(wv, f32).T).astype(np.float16)
    wpT = np.ascontiguousarray(np.asarray(wp, f32).T).astype(np.float16)

    bq = np.asarray(bq, f32)
    bk = np.asarray(bk, f32)
    bqk = np.ascontiguousarray(
        np.stack([bq.reshape(CT, P).T, bk.reshape(CT, P).T], axis=1)
    )  # [P, 2, CT]
    gnw = np.ascontiguousarray(
        np.stack(
            [np.asarray(gn_gamma, f32).reshape(CT, P).T,
             np.asarray(gn_beta, f32).reshape(CT, P).T],
            axis=2,
        )
    )  # [P, CT, 2]

    gstat = np.zeros((P, CT, G), f32)
    for t in range(CT):
        for p in range(P):
            gstat[p, t, 8 * t + p // GS] = 1.0 / GS
    gexp = np.zeros((P, CT, P), f32)  # [g(padded to 128), t, c]
    for t in range(CT):
        for c in range(P):
            gexp[8 * t + c // GS, t, c] = 1.0

    cb = (np.asarray(wp, f32) @ np.asarray(bv, f32) + np.asarray(bp, f32)).astype(f32)
    with_cb = bool(np.abs(cb).max() > 0)
    cb4 = np.ascontiguousarray(cb.reshape(CT, P).T)  # [P, CT]

    shared = {
        "wqT": wqT, "wkT": wkT, "wvT": wvT, "wpT": wpT,
        "bqk": bqk, "gnw": gnw, "gstat": gstat, "gexp": gexp,
    }
    if with_cb:
        shared["cb"] = cb4

    in_maps = []
    for b in range(B):
        m = dict(shared)
        m["x"] = np.ascontiguousarray(x[b].reshape(C, N))
        in_maps.append(m)
    return in_maps, with_cb


def kernel(x, gn_gamma, gn_beta, wq, bq, wk, bk, wv, bv, wp, bp):
    in_maps, with_cb = make_in_maps(
        x, gn_gamma, gn_beta, wq, bq, wk, bk, wv, bv, wp, bp
    )
    nc = get_nc(with_cb)
    res = run_bass_kernel_spmd(nc, in_maps, core_ids=list(range(8)))
    outs = [res.results[b]["out"].reshape(C, 64, 64) for b in range(8)]
    return np.stack(outs).astype(np.float32)
